# revision 24
# baseline (speedup 1.0000x reference)
"""Multi-head attention (B=2, N=2048, C=1024, H=16, qk-RMSNorm) on 8 TRN2 cores.

Sharding: tensor-parallel over heads x data-parallel over batch.
Core cid handles batch b = cid // 4 and head group g = cid % 4 (4 heads,
c_local = 256 channels). Each core computes qkv for its heads, per-head
RMSNorm on q/k, full softmax attention for its (b, heads), and a partial
output projection against its 256 rows of w_proj. The host sums the 4
partials per batch (TP unshard) and stacks the 2 batches.

All matmuls run as float32r (full-rate fp32, ~1.5e-4 rel err). Attention
works in transposed layout (channels on partitions): scores S^T = k @ q^T,
softmax denominators come free from a ones-column appended to V, and the
1/sum normalization is applied via a PE ones-broadcast + DVE multiply.
exp needs no max-subtraction: q/k rows are RMS-normalized so
|q.k|*scale <= sqrt(64)*sqrt(64)/8 = 8 and exp(8) is safely finite.
"""

import sys

if "/opt/trn_rl_repo" not in sys.path:
    sys.path.insert(0, "/opt/trn_rl_repo")

from contextlib import ExitStack

import numpy as np

import concourse.mybir as mybir
import concourse.tile as tile
from concourse import bacc
from concourse.bass_utils import run_bass_kernel_spmd

F32 = mybir.dt.float32
F32R = mybir.dt.float32r
BF16 = mybir.dt.bfloat16
AF = mybir.ActivationFunctionType

# Problem constants (hardcoded per contract)
B, N, C, H = 2, 2048, 1024, 16
D = C // H          # 64
EPS = 1e-6
NCORES = 8
GROUPS = 4          # head groups (cores per batch)
HL = H // GROUPS    # heads per core = 4
CL = HL * D         # local channels = 256
SCALE = D ** -0.5   # 0.125

# Tiling
P = 128             # partition dim
KT = C // P         # 8 contraction tiles over C
NQ = 512            # query-block (free dim of S^T / AV matmuls)
NB = N // P         # 16 key blocks of 128
NJ = N // NQ        # 8 query blocks per head
HPB = P // D        # heads per 128-channel block = 2
VW = D + 1          # 65: v columns + ones column


def build(n=N, nq=NQ, debug_dump=False, interleave=True, bf16_attn=False):
    """Build the SPMD Bass module. n = sequence length (for scaled tests)."""
    nb = n // P
    nj = n // nq
    kt = KT

    nc = bacc.Bacc("TRN2", target_bir_lowering=False, debug=False,
                   num_devices=NCORES)

    xT_d = nc.dram_tensor("xT", [C, n], F32, kind="ExternalInput").ap()
    wqk_d = nc.dram_tensor("w_qk", [C, 2 * CL], F32, kind="ExternalInput").ap()
    wv_d = nc.dram_tensor("w_v", [C, CL], F32, kind="ExternalInput").ap()
    wpr_d = nc.dram_tensor("w_pr", [CL, C], F32, kind="ExternalInput").ap()
    bqk_d = nc.dram_tensor("b_qk", [P, 4], F32, kind="ExternalInput").ap()
    bv_d = nc.dram_tensor("b_v", [1, CL], F32, kind="ExternalInput").ap()
    bpr_d = nc.dram_tensor("b_pr", [1, C], F32, kind="ExternalInput").ap()
    qkw_d = nc.dram_tensor("qkw", [P, 4], F32, kind="ExternalInput").ap()
    out_d = nc.dram_tensor("out", [n, C], F32, kind="ExternalOutput").ap()
    if debug_dump:
        dqk_d = nc.dram_tensor("dbg_qkT", [4, P, n], F32, kind="ExternalOutput").ap()
        dva_d = nc.dram_tensor("dbg_va", [len(range(n // P)), P, HL * VW], F32, kind="ExternalOutput").ap()
        dat_d = nc.dram_tensor("dbg_at", [CL // P, P, n], F32, kind="ExternalOutput").ap()

    with tile.TileContext(nc) as tc, ExitStack() as ctx:
        con = ctx.enter_context(tc.tile_pool(name="con", bufs=1))
        wp = ctx.enter_context(tc.tile_pool(name="wp", bufs=1))
        qk = ctx.enter_context(tc.tile_pool(name="qk", bufs=1))
        vp = ctx.enter_context(tc.tile_pool(name="vp", bufs=1))
        ps = ctx.enter_context(tc.tile_pool(name="ps", bufs=2, space="PSUM"))
        stg1 = ExitStack()
        xp = stg1.enter_context(tc.tile_pool(name="xp", bufs=1))
        sqp = stg1.enter_context(tc.tile_pool(name="sqp", bufs=1))
        rp = stg1.enter_context(tc.tile_pool(name="rp", bufs=2))

        # ---- constants ----
        ones_f = con.tile([P, P], F32, tag="onesf")
        nc.vector.memset(ones_f[:], 1.0)
        ones_c = con.tile([P, 1], F32R, tag="onesc")   # column of ones (lhsT for sumsq)
        nc.vector.tensor_copy(ones_c[:], ones_f[:, 0:1])
        ones_r = con.tile([1, P], F32R, tag="onesr")   # row of ones (lhsT for bcast/bias)
        nc.vector.tensor_copy(ones_r[:], ones_f[0:1, :])
        ones_m = con.tile([P, P], F32R, tag="onesm")    # ones matrix (lhsT for wide sumsq)
        nc.vector.tensor_copy(ones_m[:], ones_f[:])

        eps_sb = con.tile([P, 1], F32, tag="eps")
        nc.vector.memset(eps_sb[:], EPS)

        bqk_sb = con.tile([P, 4], F32, tag="bqk")
        nc.sync.dma_start(bqk_sb[:], bqk_d[:])
        qkw_sb = con.tile([P, 4], F32, tag="qkw")
        nc.sync.dma_start(qkw_sb[:], qkw_d[:])
        bv_sb = con.tile([1, CL], F32R, tag="bv")
        nc.sync.dma_start(bv_sb[:], bv_d[:].bitcast(F32R))
        bpr_sb = con.tile([1, C], F32R, tag="bpr")
        nc.sync.dma_start(bpr_sb[:], bpr_d[:].bitcast(F32R))

        # ---- weight / input loads ----
        wqk_sb = [wp.tile([P, 2 * CL], F32R, tag=f"wqk{k}", name=f"wqk{k}") for k in range(kt)]
        wv_sb = [wp.tile([P, CL], F32R, tag=f"wv{k}", name=f"wv{k}") for k in range(kt)]
        wpr_sb = [wp.tile([P, C], F32R, tag=f"wpr{k}", name=f"wpr{k}") for k in range(CL // P)]
        xT_sb = [xp.tile([P, n], F32R, tag=f"xt{k}", name=f"xt{k}") for k in range(kt)]
        for k in range(kt):
            nc.sync.dma_start(xT_sb[k][:], xT_d[k * P:(k + 1) * P, :].bitcast(F32R))
            nc.sync.dma_start(wqk_sb[k][:], wqk_d[k * P:(k + 1) * P, :].bitcast(F32R))
            nc.sync.dma_start(wv_sb[k][:], wv_d[k * P:(k + 1) * P, :].bitcast(F32R))
        for k in range(CL // P):
            nc.sync.dma_start(wpr_sb[k][:], wpr_d[k * P:(k + 1) * P, :].bitcast(F32R))

        # ---- stage 1a: qkT = (x @ w_qk)^T in [channel, n] layout ----
        # 4 channel blocks of 128: q(heads01), q(heads23), k(heads01), k(heads23)
        qpool = xp if bf16_attn else qk
        qkT = [qpool.tile([P, n], F32R, tag=f"qkT{m}", name=f"qkT{m}") for m in range(4)]
        if bf16_attn:
            qkB = [qk.tile([P, n], BF16, tag=f"qkB{m}", name=f"qkB{m}") for m in range(4)]
        for m in range(4):
            for j in range(n // 512):
                js = slice(j * 512, (j + 1) * 512)
                # split the K=128 contraction into even/odd partition halves:
                # two independent accumulation chains at PE tile rows 0 / 64
                # stream concurrently (2-for-1), summed during the copy-out
                accE = ps.tile([P, 512], F32, tag="sp", name="accE")
                accO = ps.tile([P, 512], F32, tag="ob", bufs=4, name="accO")
                for k in range(kt):
                    nc.tensor.matmul(
                        accE[:], wqk_sb[k][0:64, m * P:(m + 1) * P],
                        xT_sb[k][0:64, js], start=(k == 0), stop=(k == kt - 1))
                    nc.tensor.matmul(
                        accO[:], wqk_sb[k][64:128, m * P:(m + 1) * P],
                        xT_sb[k][64:128, js], start=(k == 0), stop=(k == kt - 1))
                tmp = rp.tile([P, 512], F32, tag="s1tmp", bufs=2, name="s1tmp")
                nc.vector.tensor_copy(tmp[:], accE[:])
                # qkT = (accO + bias_col) + accE
                nc.vector.scalar_tensor_tensor(
                    qkT[m][:, js], accO[:], bqk_sb[:, m:m + 1], tmp[:],
                    op0=mybir.AluOpType.add, op1=mybir.AluOpType.add)

        # ---- stage 1b: v natural [n, CL] with ones column per head ----
        # per key-block tile [128, HL*65]; head h at cols [h*65, h*65+65)
        v_aug = [vp.tile([P, HL * VW], BF16 if bf16_attn else F32R, tag=f"va{i}", name=f"va{i}") for i in range(nb)]
        for i in range(nb):
            acc = ps.tile([P, CL], F32, tag="sp")
            for k in range(kt):
                nc.tensor.matmul(
                    acc[:], xT_sb[k][:, i * P:(i + 1) * P], wv_sb[k][:],
                    start=(k == 0), stop=False)
            # + b_v broadcast over rows (K=1 ones matmul)
            nc.tensor.matmul(acc[:], ones_r[0:1, 0:P], bv_sb[:],
                             start=False, stop=True)
            for h in range(HL):
                nc.scalar.copy(
                    v_aug[i][:, h * VW:h * VW + D], acc[:, h * D:(h + 1) * D])
                nc.gpsimd.tensor_copy(
                    v_aug[i][:, h * VW + D:h * VW + VW], ones_f[:, 0:1])

        # ---- stage 2: per-head RMSNorm on qT, kT (in place) ----
        for m in range(4):
            for j in range(n // 512):
                js = slice(j * 512, (j + 1) * 512)
                sq = sqp.tile([P, 512], F32R, tag="sq", bufs=4)
                nc.vector.tensor_mul(sq[:], qkT[m][:, js], qkT[m][:, js])
                for h2 in range(HPB):
                    pr = slice(h2 * D, (h2 + 1) * D)
                    # sum over d broadcast to all 128 partitions in one matmul
                    ssq = ps.tile([P, 512], F32, tag="sp")
                    nc.tensor.matmul(ssq[:], ones_m[pr, :], sq[pr, :],
                                     start=True, stop=True)
                    rms = rp.tile([P, 512], F32, tag="rms")
                    nc.scalar.activation(rms[:], ssq[:], AF.Sqrt,
                                         scale=1.0 / D, bias=eps_sb[:, 0:1])
                    rec = rp.tile([P, 512], F32, tag="rec")
                    nc.vector.reciprocal_approx_fast(rec[:], rms[:])
                    # qkT = (qkT * qk_weight_col) * (1/rms)
                    dst = qkB[m] if bf16_attn else qkT[m]
                    nc.vector.scalar_tensor_tensor(
                        dst[pr, js], qkT[m][pr, js], qkw_sb[pr, m:m + 1],
                        rec[pr, :], op0=mybir.AluOpType.mult,
                        op1=mybir.AluOpType.mult)

        if debug_dump:
            for m in range(4):
                nc.sync.dma_start(dqk_d[m], qkT[m][:].bitcast(F32))
            for i in range(nb):
                nc.sync.dma_start(dva_d[i], v_aug[i][:].bitcast(F32))

        # ---- stage 3 + 4: attention per (query block, head), then projection ----
        # xT / stage-1+2 scratch are dead now; release their SBUF for the
        # attention-stage pools
        stg1.close()
        ptp = ctx.enter_context(tc.tile_pool(name="ptp", bufs=20))
        atp = ctx.enter_context(tc.tile_pool(name="atp", bufs=1))
        rp2 = ctx.enter_context(tc.tile_pool(name="rp2", bufs=4))
        osp = ctx.enter_context(tc.tile_pool(name="osp", bufs=4))
        attnT = [atp.tile([P, n], F32R, tag=f"at{t}", name=f"at{t}") for t in range(CL // P)]
        # units are (query-block, head-PAIR): the even head's kT/qT live at
        # partition offset 0, the odd head's at offset 64, so consecutive S
        # matmuls target alternating PE array tiles (row 0 / row 64) and
        # stream concurrently (~2x single-position throughput)
        units = [(j, hp) for j in range(nj) for hp in range(HL // HPB)]

        def emit_s_pair(u, i):
            """S^T matmuls for both heads of the pair at key-block i + exp."""
            j, hp = u
            js = slice(j * nq, (j + 1) * nq)
            qm, km = hp, 2 + hp
            qsrc = qkB if bf16_attn else qkT
            s2 = ps.tile([P, 2 * nq], F32, tag="sp", name="s2")
            for sub in range(HPB):
                pr = slice(sub * D, (sub + 1) * D)
                nc.tensor.matmul(
                    s2[:, sub * nq:(sub + 1) * nq],
                    qsrc[km][pr, i * P:(i + 1) * P], qsrc[qm][pr, js],
                    start=True, stop=True)
            pt = ptp.tile([P, 2 * nq], BF16 if bf16_attn else F32R, tag="pt", name="pt")
            nc.scalar.activation(pt[:], s2[:], AF.Exp, scale=SCALE)
            return pt

        def emit_av(u, oas, pts, i):
            j, hp = u
            for sub in range(HPB):
                h = hp * HPB + sub
                nc.tensor.matmul(
                    oas[sub][:], v_aug[i][:, h * VW:(h + 1) * VW],
                    pts[i][:, sub * nq:(sub + 1) * nq],
                    start=(i == 0), stop=(i == nb - 1))

        def emit_norm(u, oas):
            j, hp = u
            js = slice(j * nq, (j + 1) * nq)
            for sub in range(HPB):
                h = hp * HPB + sub
                oa = oas[sub]
                sums_sb = rp2.tile([1, nq], F32R, tag="sums", name="sums")
                nc.vector.tensor_copy(sums_sb[:], oa[D:VW, :])
                bc = ps.tile([D, nq], F32, tag="ob", bufs=4, name="bc")
                nc.tensor.matmul(bc[:], ones_r[0:1, 0:D], sums_sb[:],
                                 start=True, stop=True)
                rec = rp2.tile([D, nq], F32, tag="recw", name="recw")
                nc.vector.reciprocal_approx_fast(rec[:], bc[:])
                t = (h * D) // P
                prA = slice((h * D) % P, (h * D) % P + D)
                nc.vector.tensor_mul(attnT[t][prA, js], oa[0:D, :], rec[:])

        def emit_proj(j):
            for j2 in range(j * (nq // P), (j + 1) * (nq // P)):
                for half in range(2):
                    hs = slice(half * 512, (half + 1) * 512)
                    acc = ps.tile([P, 512], F32, tag="ob", bufs=4, name="acc")
                    for t in range(CL // P):
                        nc.tensor.matmul(
                            acc[:], attnT[t][:, j2 * P:(j2 + 1) * P],
                            wpr_sb[t][:, hs], start=(t == 0), stop=False)
                    nc.tensor.matmul(acc[:], ones_r[0:1, 0:P], bpr_sb[:, hs],
                                     start=False, stop=True)
                    ost = osp.tile([P, 512], F32, tag="ost", name="ost")
                    nc.vector.tensor_copy(ost[:], acc[:])
                    nc.sync.dma_start(out_d[j2 * P:(j2 + 1) * P, hs], ost[:])

        # software pipeline: S pairs of unit u+1 interleave with AV of unit u
        prev = None   # (unit, pts)
        for idx in range(len(units) + 1):
            cur = units[idx] if idx < len(units) else None
            pts = []
            oas_prev = None
            if prev is not None:
                oas_prev = [ps.tile([VW, nq], F32, tag="ob", bufs=4,
                                    name=f"oa{s_}") for s_ in range(HPB)]
            BLK = 4
            for ib in range(nb // BLK):
                if cur is not None:
                    for i in range(ib * BLK, (ib + 1) * BLK):
                        pts.append(emit_s_pair(cur, i))
                if prev is not None:
                    for i in range(ib * BLK, (ib + 1) * BLK):
                        emit_av(prev[0], oas_prev, prev[1], i)
            if prev is not None:
                emit_norm(prev[0], oas_prev)
                jprev, hpprev = prev[0]
                if hpprev == HL // HPB - 1:
                    emit_proj(jprev)
            prev = (cur, pts) if cur is not None else None

        if debug_dump:
            for t in range(CL // P):
                nc.sync.dma_start(dat_d[t], attnT[t][:].bitcast(F32))

    nc.compile()
    return nc


_NC_CACHE = {}


def _get_nc(n=N, nq=NQ):
    key = (n, nq)
    if key not in _NC_CACHE:
        _NC_CACHE[key] = build(n, nq)
    return _NC_CACHE[key]


def make_in_maps(x, w_qkv, b_qkv, q_w, k_w, w_proj, b_proj):
    """Shard full inputs into per-core in_maps (host side)."""
    n = x.shape[1]
    in_maps = []
    for cid in range(NCORES):
        b, g = cid // GROUPS, cid % GROUPS
        c0 = g * CL
        xT = np.ascontiguousarray(x[b].T)
        w_qk = np.ascontiguousarray(
            np.concatenate([w_qkv[:, c0:c0 + CL],
                            w_qkv[:, C + c0:C + c0 + CL]], axis=1))
        w_v = np.ascontiguousarray(w_qkv[:, 2 * C + c0:2 * C + c0 + CL])
        w_pr = np.ascontiguousarray(w_proj[c0:c0 + CL, :])
        b_qk = np.stack([b_qkv[c0 + m * P:c0 + (m + 1) * P] for m in range(2)]
                        + [b_qkv[C + c0 + m * P:C + c0 + (m + 1) * P]
                           for m in range(2)], axis=1)
        b_v = b_qkv[2 * C + c0:2 * C + c0 + CL].reshape(1, CL)
        # host gather sums GROUPS partials per batch; split the bias so it
        # lands exactly once
        b_pr = (b_proj / GROUPS).reshape(1, C)
        qkw = np.stack([np.tile(q_w, HPB), np.tile(q_w, HPB),
                        np.tile(k_w, HPB), np.tile(k_w, HPB)], axis=1)
        in_maps.append({
            "xT": xT.astype(np.float32),
            "w_qk": w_qk.astype(np.float32),
            "w_v": w_v.astype(np.float32),
            "w_pr": w_pr.astype(np.float32),
            "b_qk": np.ascontiguousarray(b_qk).astype(np.float32),
            "b_v": b_v.astype(np.float32),
            "b_pr": b_pr.astype(np.float32),
            "qkw": np.ascontiguousarray(qkw).astype(np.float32),
        })
    return in_maps


def kernel(x, w_qkv, b_qkv, q_w, k_w, w_proj, b_proj, _trace=False):
    x = np.asarray(x)
    n = x.shape[1]
    nc = _get_nc(n, NQ if n % NQ == 0 else P)
    in_maps = make_in_maps(np.asarray(x, np.float32), np.asarray(w_qkv, np.float32),
                           np.asarray(b_qkv, np.float32), np.asarray(q_w, np.float32),
                           np.asarray(k_w, np.float32), np.asarray(w_proj, np.float32),
                           np.asarray(b_proj, np.float32))
    res = run_bass_kernel_spmd(nc, in_maps, core_ids=list(range(NCORES)),
                               trace=_trace)
    # TP unshard: sum the 4 head-group partials per batch, stack batches
    out = np.stack([
        sum(res.results[b * GROUPS + g]["out"] for g in range(GROUPS))
        for b in range(B)
    ]).astype(np.float32)
    if _trace:
        return out, res
    return out
